# revision 1
# baseline (speedup 1.0000x reference)
"""Causal self-attention (B=2, T=2048, C=1024, H=16, D=64) on 8 trn2 NeuronCores.

Sharding: core c = (batch b = c//4) x (head-group g = c%4; heads 4g..4g+3).
Tensor-parallel on heads for qkv_proj (column split) / out_proj (row split),
data-parallel on batch. Each core computes a full [C, T] partial of the
output projection for its batch; the host sums the 4 head-group partials
per batch and transposes back to [T, C].

Device kernel (per core), all matmul operands float32r (tf32-like):
  1. QKV: W-stationary matmuls -> Q^T, K^T, V^T in [n, t] layout (+bias).
  2. PE-transpose V^T -> V_aug tiles [t_tile][128, 4*(64+1)] (ones column
     appended per head for the softmax row-sum).
  3. Attention in S^T layout: S^T tile = K_h @ Q_h^T (128tk x 512tq),
     exp(x/8) on ACT, causal mask multiply on the 4 diagonal-crossing
     tiles, then U^T += V_aug^T @ P^T accumulated over tk tiles; row 64 of
     the psum accumulates the softmax denominator l.
  4. Normalize: r = 1/l (DVE), partition-broadcast r (GPSIMD), multiply.
  5. Out-proj: W_out-stationary matmuls -> out^T [1024, 2048] + bias.
"""

import sys

if "/opt/trn_rl_repo" not in sys.path:
    sys.path.insert(0, "/opt/trn_rl_repo")

import numpy as np

B, T, C = 2, 2048, 1024
H, D = 16, 64
HPC = 4            # heads per core
NC_ = HPC * D      # 256 qkv columns per core per projection
N_CORES = 8
PT = 128           # partition tile
TT = T // PT       # 16 t tiles
QC = 512           # tq chunk (moving free dim)
NQC = T // QC      # 4 tq chunks
KC = C // PT       # 8 contraction chunks for qkv proj
MQKV = 3 * NC_ // PT  # 6 output row chunks of qkv proj
VA = D + 1         # v_aug cols per head

_CACHE = {}
_DEBUG = False


def _build_nc():
    import concourse.bacc as bacc
    import concourse.mybir as mybir
    import concourse.tile as tile
    from contextlib import ExitStack

    f32 = mybir.dt.float32
    f32r = mybir.dt.float32r
    Act = mybir.ActivationFunctionType

    nc = bacc.Bacc("TRN2", target_bir_lowering=False, debug=False,
                   num_devices=N_CORES)

    xT_d = nc.dram_tensor("xT", [C, T], f32, kind="ExternalInput").ap()
    wqkv_d = nc.dram_tensor("wqkv", [C, 3 * NC_], f32, kind="ExternalInput").ap()
    bqkv_d = nc.dram_tensor("bqkv", [3 * NC_, 1], f32, kind="ExternalInput").ap()
    wout_d = nc.dram_tensor("wout", [NC_, C], f32, kind="ExternalInput").ap()
    bout_d = nc.dram_tensor("bout", [C, 1], f32, kind="ExternalInput").ap()
    trimask_d = nc.dram_tensor("trimask", [PT, PT], f32, kind="ExternalInput").ap()
    ones4_d = nc.dram_tensor("ones4", [PT, HPC], f32, kind="ExternalInput").ap()
    ident_d = nc.dram_tensor("ident", [PT, PT], f32, kind="ExternalInput").ap()
    outT_d = nc.dram_tensor("outT", [C, T], f32, kind="ExternalOutput").ap()
    if _DEBUG:
        dbgqk_d = nc.dram_tensor("dbgqk", [4 * PT, T], f32, kind="ExternalOutput").ap()
        dbgva_d = nc.dram_tensor("dbgva", [TT * PT, HPC * VA], f32, kind="ExternalOutput").ap()
        dbgun_d = nc.dram_tensor("dbgun", [2 * PT, T], f32, kind="ExternalOutput").ap()

    with tile.TileContext(nc) as tc, ExitStack() as ctx:
        p_xt = ctx.enter_context(tc.tile_pool(name="xt", bufs=KC))
        p_wq = ctx.enter_context(tc.tile_pool(name="wq", bufs=KC))
        p_qk = ctx.enter_context(tc.tile_pool(name="qk", bufs=4))
        p_vt = ctx.enter_context(tc.tile_pool(name="vt", bufs=4))
        p_va = ctx.enter_context(tc.tile_pool(name="va", bufs=TT))
        p_wo = ctx.enter_context(tc.tile_pool(name="wo", bufs=2))
        p_small = ctx.enter_context(tc.tile_pool(name="small", bufs=1))
        p_pt = ctx.enter_context(tc.tile_pool(name="ptile", bufs=3))
        p_norm = ctx.enter_context(tc.tile_pool(name="norm", bufs=2))
        p_out = ctx.enter_context(tc.tile_pool(name="outs", bufs=2))
        ps_1 = ctx.enter_context(tc.tile_pool(name="ps1", bufs=2, space="PSUM"))
        ps_2 = ctx.enter_context(tc.tile_pool(name="ps2", bufs=2, space="PSUM"))

        # ---- loads -------------------------------------------------------
        wq_sb = []
        for k in range(KC):
            w = p_wq.tile([PT, 3 * NC_], f32r, tag="wq")
            nc.sync.dma_start(w[:], wqkv_d[k * PT:(k + 1) * PT, :].bitcast(f32r))
            wq_sb.append(w)

        bq_sb = []
        for m in range(MQKV):
            b = p_small.tile([PT, 1], f32, tag="bq", bufs=MQKV)
            nc.sync.dma_start(b[:], bqkv_d[m * PT:(m + 1) * PT, :])
            bq_sb.append(b)

        # x^T column-chunked loads so qkv matmuls can start early
        xt_sb = [p_xt.tile([PT, T], f32r, tag="xt", name=f"xt{k}") for k in range(KC)]
        for cq in range(NQC):
            for k in range(KC):
                nc.sync.dma_start(
                    xt_sb[k][:, cq * QC:(cq + 1) * QC],
                    xT_d[k * PT:(k + 1) * PT, cq * QC:(cq + 1) * QC].bitcast(f32r),
                )

        trimask = p_small.tile([PT, PT], f32r, tag="trimask")
        nc.sync.dma_start(trimask[:], trimask_d[:].bitcast(f32r))
        ident = p_small.tile([PT, PT], f32r, tag="ident")
        nc.sync.dma_start(ident[:], ident_d[:].bitcast(f32r))
        wo_sb = []
        for k in range(2):
            w = p_wo.tile([PT, C], f32r, tag="wo")
            nc.sync.dma_start(w[:], wout_d[k * PT:(k + 1) * PT, :].bitcast(f32r))
            wo_sb.append(w)
        bo_sb = []
        for e in range(C // PT):
            b = p_small.tile([PT, 1], f32, tag="bo", bufs=C // PT)
            nc.sync.dma_start(b[:], bout_d[e * PT:(e + 1) * PT, :])
            bo_sb.append(b)

        # ---- phase 1: qkv projection -> Q^T, K^T (packed), V_aug --------
        # m chunk 0..1 -> Q^T rows, 2..3 -> K^T rows, 4..5 -> V^T rows
        qk_sb = [p_qk.tile([PT, T], f32r, tag="qk", name=f"qk{j}") for j in range(4)]
        va_sb = [p_va.tile([PT, HPC * VA], f32r, tag="va", name=f"va{t}") for t in range(TT)]

        for cq in range(NQC):
            cs = slice(cq * QC, (cq + 1) * QC)
            for m in range(MQKV):
                ps = ps_1.tile([PT, QC], f32, tag="sa")
                for k in range(KC):
                    nc.tensor.matmul(
                        ps[:],
                        wq_sb[k][:, m * PT:(m + 1) * PT],
                        xt_sb[k][:, cs],
                        start=(k == 0), stop=(k == KC - 1),
                    )
                if m < 4:
                    nc.vector.tensor_scalar_add(qk_sb[m][:, cs], ps[:], bq_sb[m])
                else:
                    h0, h1 = 2 * (m - 4), 2 * (m - 4) + 1
                    for q4 in range(4):
                        t = cq * 4 + q4
                        vtp = p_vt.tile([PT, PT], f32r, tag="vt")
                        nc.vector.tensor_scalar_add(
                            vtp[:], ps[:, q4 * PT:(q4 + 1) * PT], bq_sb[m])
                        pst = ps_1.tile([PT, PT], f32r, tag="sa")
                        nc.tensor.transpose(pst[:], vtp[:], ident[:])
                        nc.vector.tensor_copy(
                            va_sb[t][:, h0 * VA:h0 * VA + D], pst[:, 0:D])
                        nc.vector.tensor_copy(
                            va_sb[t][:, h1 * VA:h1 * VA + D], pst[:, D:2 * D])

        # ones columns for the softmax row-sum (gpsimd queue; off the SP path)
        for t in range(TT):
            nc.gpsimd.dma_start(
                va_sb[t].rearrange("p (h v) -> p h v", v=VA)[:, :, D:D + 1],
                ones4_d[:].bitcast(f32r),
            )

        # ---- phase 2+3: attention (head pairs) + interleaved out-proj ----
        # UN tiles reuse xt pool slots (x^T fully consumed by phase 1)
        un_sb = [p_xt.tile([PT, T], f32r, tag="xt", name=f"un{j}") for j in range(2)]

        for cq in range(NQC):
            cs = slice(cq * QC, (cq + 1) * QC)
            nts = 4 * cq + 4
            for j in range(2):          # head pair (2j, 2j+1)
                h0, h1 = 2 * j, 2 * j + 1
                psu = ps_2.tile([PT, 2 * QC], f32, tag="acc")
                for t in range(nts):
                    p = t - 4 * cq      # >= 0 on diagonal-crossing tiles
                    s = max(p, 0) * PT  # skip fully-masked leading columns
                    w = QC - s
                    psS = ps_1.tile([PT, 2 * QC], f32, tag="sa")
                    tsl = slice(t * PT, (t + 1) * PT)
                    qsl = slice(cq * QC + s, (cq + 1) * QC)
                    nc.tensor.matmul(
                        psS[:, s:QC],
                        qk_sb[2 + j][0:D, tsl], qk_sb[j][0:D, qsl],
                        start=True, stop=True, tile_position=(0, 0),
                    )
                    nc.tensor.matmul(
                        psS[:, QC + s:2 * QC],
                        qk_sb[2 + j][D:PT, tsl], qk_sb[j][D:PT, qsl],
                        start=True, stop=True, tile_position=(D, 0),
                    )
                    pt = p_pt.tile([PT, 2 * QC], f32r, tag="pt")
                    pt3 = pt.rearrange("p (h w) -> p h w", h=2)
                    psS3 = psS.rearrange("p (h w) -> p h w", h=2)
                    nc.scalar.activation(pt3[:, :, s:QC], psS3[:, :, s:QC],
                                         Act.Exp, scale=0.125)
                    if p >= 0:
                        nc.vector.tensor_mul(
                            pt[:, s:s + PT], pt[:, s:s + PT], trimask[:])
                        nc.vector.tensor_mul(
                            pt[:, QC + s:QC + s + PT], pt[:, QC + s:QC + s + PT],
                            trimask[:])
                    nc.tensor.matmul(
                        psu[0:VA, s:QC],
                        va_sb[t][:, h0 * VA:(h0 + 1) * VA], pt[:, s:QC],
                        start=(t == 0), stop=(t == nts - 1),
                    )
                    nc.tensor.matmul(
                        psu[0:VA, QC + s:2 * QC],
                        va_sb[t][:, h1 * VA:(h1 + 1) * VA], pt[:, QC + s:2 * QC],
                        start=(t == 0), stop=(t == nts - 1),
                    )
                # normalize: rows 0..63 = U^T, row 64 = l (both heads)
                rr = p_norm.tile([VA, 2 * QC], f32, tag="rr")
                nc.vector.reciprocal(rr[D:VA, :], psu[D:VA, :])
                rb = p_norm.tile([D, 2 * QC], f32, tag="rb")
                # shift 1/l from partition 64 to partition 0, then broadcast
                # (partition_broadcast ucode reads physical partition 0)
                nc.gpsimd.dma_start(rb[0:1, :], rr[D:VA, :])
                nc.gpsimd.partition_broadcast(rb[0:D, :], rb[0:1, :])
                nc.vector.tensor_mul(un_sb[j][0:D, cs], psu[0:D, 0:QC],
                                     rb[:, 0:QC])
                ut = p_norm.tile([D, QC], f32r, tag="ut")
                nc.vector.tensor_mul(ut[:], psu[0:D, QC:2 * QC], rb[:, QC:2 * QC])
                nc.gpsimd.dma_start(un_sb[j][D:PT, cs], ut[:])

            # out-projection for this tq chunk (e-chunk pairs)
            for ep in range(4):
                pp2 = ps_1.tile([PT, 2 * QC], f32, tag="sa")
                for half in range(2):
                    e = 2 * ep + half
                    for k in range(2):
                        nc.tensor.matmul(
                            pp2[:, half * QC:(half + 1) * QC],
                            wo_sb[k][:, e * PT:(e + 1) * PT],
                            un_sb[k][:, cs],
                            start=(k == 0), stop=(k == 1),
                        )
                ot = p_out.tile([PT, 2 * QC], f32, tag="ot")
                for half in range(2):
                    e = 2 * ep + half
                    nc.vector.tensor_scalar_add(
                        ot[:, half * QC:(half + 1) * QC],
                        pp2[:, half * QC:(half + 1) * QC], bo_sb[e])
                    nc.sync.dma_start(
                        outT_d[e * PT:(e + 1) * PT, cs],
                        ot[:, half * QC:(half + 1) * QC])

        if _DEBUG:
            for j in range(4):
                nc.sync.dma_start(dbgqk_d[j * PT:(j + 1) * PT, :],
                                  qk_sb[j][:].bitcast(f32))
            for t in range(TT):
                nc.sync.dma_start(dbgva_d[t * PT:(t + 1) * PT, :],
                                  va_sb[t][:].bitcast(f32))
            for j in range(2):
                nc.sync.dma_start(dbgun_d[j * PT:(j + 1) * PT, :],
                                  un_sb[j][:].bitcast(f32))

    nc.compile()
    return nc


def _get_nc():
    if "nc" not in _CACHE:
        _CACHE["nc"] = _build_nc()
    return _CACHE["nc"]


def _make_in_maps(x, W_qkv, b_qkv, W_out, b_out):
    x = np.asarray(x, dtype=np.float32)
    W_qkv = np.asarray(W_qkv, dtype=np.float32)
    b_qkv = np.asarray(b_qkv, dtype=np.float32)
    W_out = np.asarray(W_out, dtype=np.float32)
    b_out = np.asarray(b_out, dtype=np.float32)

    i = np.arange(PT)[:, None]
    j = np.arange(PT)[None, :]
    trimask = (i <= j).astype(np.float32)
    ones4 = np.ones((PT, HPC), dtype=np.float32)
    ident = np.eye(PT, dtype=np.float32)

    in_maps = []
    for c in range(N_CORES):
        b, g = divmod(c, 4)
        gs = slice(g * NC_, (g + 1) * NC_)
        wqkv_c = np.ascontiguousarray(np.concatenate(
            [W_qkv[:, gs], W_qkv[:, C:][:, gs], W_qkv[:, 2 * C:][:, gs]],
            axis=1))
        bqkv_c = np.ascontiguousarray(np.concatenate(
            [b_qkv[gs], b_qkv[C:][gs], b_qkv[2 * C:][gs]])[:, None])
        bout_c = (b_out if g == 0 else np.zeros_like(b_out))[:, None]
        in_maps.append({
            "xT": np.ascontiguousarray(x[b].T),
            "wqkv": wqkv_c,
            "bqkv": bqkv_c,
            "wout": np.ascontiguousarray(W_out[gs, :]),
            "bout": np.ascontiguousarray(bout_c),
            "trimask": trimask,
            "ones4": ones4,
            "ident": ident,
        })
    return in_maps


def _assemble(results):
    out = np.empty((B, T, C), dtype=np.float32)
    for b in range(B):
        acc = results[4 * b]["outT"].copy()
        for g in range(1, 4):
            acc += results[4 * b + g]["outT"]
        out[b] = acc.T
    return out


def kernel(x, W_qkv, b_qkv, W_out, b_out):
    from concourse import bass_utils
    nc = _get_nc()
    in_maps = _make_in_maps(x, W_qkv, b_qkv, W_out, b_out)
    res = bass_utils.run_bass_kernel_spmd(nc, in_maps, core_ids=list(range(N_CORES)))
    return _assemble(res.results)



# revision 6
# speedup vs baseline: 1.5379x; 1.5379x over previous
"""Causal self-attention (B=2, T=2048, C=1024, H=16, D=64) on 8 trn2 NeuronCores.

Sharding: core c = (batch b = c//4) x (head-group g = c%4; heads 4g..4g+3).
Tensor-parallel on heads for qkv_proj (column split) / out_proj (row split),
data-parallel on batch. Each core computes a full [C, T] partial of the
output projection for its batch; the host sums the 4 head-group partials
per batch and transposes back to [T, C].

v2: bf16 matmul datapath (fp32 PSUM accumulate), batched input DMAs with
host-side repacking, 4-deep QKV psum rotation, software-pipelined
attention (S(t+1) issued before exp/PV(t)), out-proj deferred one tq
chunk to fill the PE during the normalize chain, and a cheap normalize:
l-row copy (DVE) -> partition shift (gpsimd DMA) -> rank-1 broadcast
matmul (PE) -> reciprocal_approx_fast (DVE) -> scale (DVE).
"""

import sys

if "/opt/trn_rl_repo" not in sys.path:
    sys.path.insert(0, "/opt/trn_rl_repo")

import numpy as np

B, T, C = 2, 2048, 1024
H, D = 16, 64
HPC = 4            # heads per core
NC_ = HPC * D      # 256 qkv columns per core per projection
N_CORES = 8
PT = 128           # partition tile
TT = T // PT       # 16 t tiles
QC = 512           # tq chunk (moving free dim)
NQC = T // QC      # 4 tq chunks
KC = C // PT       # 8 contraction chunks for qkv proj
MQKV = 3 * NC_ // PT  # 6 output row chunks of qkv proj
VA = D + 1         # v_aug cols per head

_CACHE = {}


def _build_nc():
    import concourse.bacc as bacc
    import concourse.mybir as mybir
    import concourse.tile as tile
    from contextlib import ExitStack

    f32 = mybir.dt.float32
    f32r = mybir.dt.float32r
    bf16 = mybir.dt.bfloat16
    Act = mybir.ActivationFunctionType

    nc = bacc.Bacc("TRN2", target_bir_lowering=False, debug=False,
                   num_devices=N_CORES)

    # Host-repacked inputs (see _make_in_maps):
    #   xr:  [128, NQC*KC*512] bf16 — x^T chunked [p, cq, k, 512]
    #   wq:  [128, KC*768]     bf16 — W_qkv chunk rows side by side
    #   wo:  [128, 2*1024]     bf16 — W_out chunk rows side by side
    xr_d = nc.dram_tensor("xr", [PT, NQC * KC * QC], bf16, kind="ExternalInput").ap()
    wq_d = nc.dram_tensor("wq", [PT, KC * 3 * NC_], bf16, kind="ExternalInput").ap()
    wo_d = nc.dram_tensor("wo", [PT, 2 * C], bf16, kind="ExternalInput").ap()
    bq_d = nc.dram_tensor("bq", [PT, MQKV], f32, kind="ExternalInput").ap()
    bo_d = nc.dram_tensor("bo", [PT, C // PT], f32, kind="ExternalInput").ap()
    trimask_d = nc.dram_tensor("trimask", [PT, PT], bf16, kind="ExternalInput").ap()
    ident_d = nc.dram_tensor("ident", [PT, PT], bf16, kind="ExternalInput").ap()
    ones64_d = nc.dram_tensor("ones64", [1, D], f32, kind="ExternalInput").ap()
    outT_d = nc.dram_tensor("outT", [C, T], f32, kind="ExternalOutput").ap()

    with tile.TileContext(nc) as tc, ExitStack() as ctx:
        p_big = ctx.enter_context(tc.tile_pool(name="big", bufs=1))
        p_qk = ctx.enter_context(tc.tile_pool(name="qk", bufs=4))
        p_vt = ctx.enter_context(tc.tile_pool(name="vt", bufs=4))
        p_va = ctx.enter_context(tc.tile_pool(name="va", bufs=TT))
        p_small = ctx.enter_context(tc.tile_pool(name="small", bufs=1))
        p_pt = ctx.enter_context(tc.tile_pool(name="ptile", bufs=3))
        p_norm = ctx.enter_context(tc.tile_pool(name="norm", bufs=2))
        p_out = ctx.enter_context(tc.tile_pool(name="outs", bufs=2))
        ps_a = ctx.enter_context(tc.tile_pool(name="psa", bufs=2, space="PSUM"))
        ps_u = ctx.enter_context(tc.tile_pool(name="psu", bufs=2, space="PSUM"))

        # ---- loads (few, large DMAs; issue order = need order) ----------
        wq_sb = p_big.tile([PT, KC * 3 * NC_], bf16, tag="wq")
        nc.sync.dma_start(wq_sb[:, :KC * 3 * NC_ // 2], wq_d[:, :KC * 3 * NC_ // 2])
        nc.sync.dma_start(wq_sb[:, KC * 3 * NC_ // 2:], wq_d[:, KC * 3 * NC_ // 2:])
        bq_sb = p_small.tile([PT, MQKV], f32, tag="bq")
        nc.sync.dma_start(bq_sb[:], bq_d[:])

        # x^T: SBUF layout [p, k, t] (k-major); DRAM layout [p, cq, k, 512]
        xt_sb = p_big.tile([PT, KC * T], bf16, tag="xt")
        xt3 = xt_sb.rearrange("p (k t) -> p k t", t=T)
        xr3 = xr_d.rearrange("p (ck q) -> p ck q", q=QC)
        for cq in range(NQC):
            nc.sync.dma_start(
                xt3[:, :, cq * QC:(cq + 1) * QC],
                xr3[:, cq * KC:(cq + 1) * KC, :],
            )

        ident = p_small.tile([PT, PT], bf16, tag="ident")
        nc.scalar.dma_start(ident[:], ident_d[:])
        trimask = p_small.tile([PT, PT], bf16, tag="trimask")
        nc.scalar.dma_start(trimask[:], trimask_d[:])
        ones64 = p_small.tile([1, D], f32r, tag="ones64")
        nc.scalar.dma_start(ones64[:], ones64_d[:].bitcast(f32r))
        wo_sb = p_big.tile([PT, 2 * C], bf16, tag="wo")
        nc.scalar.dma_start(wo_sb[:], wo_d[:])
        bo_sb = p_small.tile([PT, C // PT], f32, tag="bo")
        nc.scalar.dma_start(bo_sb[:], bo_d[:])

        # ---- phase 1: qkv projection -> Q^T, K^T (packed, bf16), V_aug --
        # m chunk 0..1 -> Q^T rows, 2..3 -> K^T rows, 4..5 -> V^T rows
        qk_sb = [p_qk.tile([PT, T], bf16, tag="qk", name=f"qk{j}") for j in range(4)]
        va_sb = [p_va.tile([PT, HPC * VA], bf16, tag="va", name=f"va{t}")
                 for t in range(TT)]
        for t in range(TT):
            nc.gpsimd.memset(
                va_sb[t].rearrange("p (h v) -> p h v", v=VA)[:, :, D:D + 1], 1.0)

        # QKV psums rotate through the same slots the attention phase uses
        # (ps_a tag "sa" / ps_u tag "acc"), giving a 4-deep rotation.
        rot = [(ps_a, "sa"), (ps_u, "acc")]
        ri = 0

        def alloc_ps(shape, name, dtype=f32):
            nonlocal ri
            pool, tag = rot[ri % 2]
            ri += 1
            return pool.tile(shape, dtype, tag=tag, name=name)

        for cq in range(NQC):
            cs = slice(cq * QC, (cq + 1) * QC)
            for m in range(MQKV):
                ps = alloc_ps([PT, QC], "ps")
                for k in range(KC):
                    nc.tensor.matmul(
                        ps[:],
                        wq_sb[:, k * 3 * NC_ + m * PT:k * 3 * NC_ + (m + 1) * PT],
                        xt3[:, k, cs],
                        start=(k == 0), stop=(k == KC - 1),
                    )
                if m < 4:
                    nc.vector.tensor_scalar_add(qk_sb[m][:, cs], ps[:],
                                                bq_sb[:, m:m + 1])
                else:
                    h0 = 2 * (m - 4)
                    for q4 in range(4):
                        t = cq * 4 + q4
                        vtp = p_vt.tile([PT, PT], bf16, tag="vt")
                        nc.vector.tensor_scalar_add(
                            vtp[:], ps[:, q4 * PT:(q4 + 1) * PT],
                            bq_sb[:, m:m + 1])
                        pst = alloc_ps([PT, PT], "pst", bf16)
                        nc.tensor.transpose(pst[:], vtp[:], ident[:])
                        nc.vector.tensor_copy(
                            va_sb[t].rearrange("p (h v) -> p h v", v=VA)
                            [:, h0:h0 + 2, 0:D],
                            pst.rearrange("p (h v) -> p h v", v=D)[:, 0:2, :])

        # ---- phase 2+3: attention (head pairs) + deferred out-proj ------
        un_sb = [p_qk.tile([PT, T], bf16, tag="un", bufs=2, name=f"un{j}")
                 for j in range(2)]

        def emit_outproj(cq):
            cs = slice(cq * QC, (cq + 1) * QC)
            for ep in range(4):
                pp2 = ps_a.tile([PT, 2 * QC], f32, tag="sa", name="pp2")
                for half in range(2):
                    e = 2 * ep + half
                    for k in range(2):
                        nc.tensor.matmul(
                            pp2[:, half * QC:(half + 1) * QC],
                            wo_sb[:, k * C + e * PT:k * C + (e + 1) * PT],
                            un_sb[k][:, cs],
                            start=(k == 0), stop=(k == 1),
                        )
                ot = p_out.tile([PT, 2 * QC], f32, tag="ot")
                for half in range(2):
                    e = 2 * ep + half
                    nc.vector.tensor_scalar_add(
                        ot[:, half * QC:(half + 1) * QC],
                        pp2[:, half * QC:(half + 1) * QC], bo_sb[:, e:e + 1])
                    nc.sync.dma_start(
                        outT_d[e * PT:(e + 1) * PT, cs],
                        ot[:, half * QC:(half + 1) * QC])

        for cq in range(NQC):
            cs = slice(cq * QC, (cq + 1) * QC)
            nts = 4 * cq + 4
            psus = [None, None]
            for j in range(2):          # head pair (2j, 2j+1)
                h0, h1 = 2 * j, 2 * j + 1
                psu = ps_u.tile([PT, 2 * QC], f32, tag="acc", name="psu")
                psus[j] = psu

                stage = []  # pending (t, psS) awaiting exp/mask/PV

                def drain(j=j, h0=h0, h1=h1, psu=psu, nts=nts, cq=cq):
                    t, psS = stage.pop(0)
                    p = t - 4 * cq
                    s = max(p, 0) * PT
                    pt = p_pt.tile([PT, 2 * QC], bf16, tag="pt")
                    pt3 = pt.rearrange("p (h w) -> p h w", h=2)
                    psS3 = psS.rearrange("p (h w) -> p h w", h=2)
                    nc.scalar.activation(pt3[:, :, s:QC], psS3[:, :, s:QC],
                                         Act.Exp, scale=0.125)
                    if p >= 0:
                        nc.vector.tensor_mul(
                            pt[:, s:s + PT], pt[:, s:s + PT], trimask[:])
                        nc.vector.tensor_mul(
                            pt[:, QC + s:QC + s + PT],
                            pt[:, QC + s:QC + s + PT], trimask[:])
                    nc.tensor.matmul(
                        psu[0:VA, s:QC],
                        va_sb[t][:, h0 * VA:(h0 + 1) * VA], pt[:, s:QC],
                        start=(t == 0), stop=(t == nts - 1),
                    )
                    nc.tensor.matmul(
                        psu[0:VA, QC + s:2 * QC],
                        va_sb[t][:, h1 * VA:(h1 + 1) * VA],
                        pt[:, QC + s:2 * QC],
                        start=(t == 0), stop=(t == nts - 1),
                    )

                for t in range(nts):
                    p = t - 4 * cq      # >= 0 on diagonal-crossing tiles
                    s = max(p, 0) * PT  # skip fully-masked leading columns
                    psS = ps_a.tile([PT, 2 * QC], f32, tag="sa", name="psS")
                    tsl = slice(t * PT, (t + 1) * PT)
                    qsl = slice(cq * QC + s, (cq + 1) * QC)
                    nc.tensor.matmul(
                        psS[:, s:QC],
                        qk_sb[2 + j][0:D, tsl], qk_sb[j][0:D, qsl],
                        start=True, stop=True, tile_position=(0, 0),
                    )
                    nc.tensor.matmul(
                        psS[:, QC + s:2 * QC],
                        qk_sb[2 + j][D:PT, tsl], qk_sb[j][D:PT, qsl],
                        start=True, stop=True, tile_position=(D, 0),
                    )
                    stage.append((t, psS))
                    if len(stage) > 1:
                        drain()
                drain()

            # normalize j=0: U^T rows 0..63, l in row 64 (both heads)
            def emit_norm(j, psu):
                rr = p_norm.tile([VA, 2 * QC], f32, tag="rr")
                nc.vector.tensor_copy(rr[D:VA, :], psu[D:VA, :])
                rsh = p_norm.tile([1, 2 * QC], f32, tag="rsh")
                nc.gpsimd.dma_start(rsh[0:1, :], rr[D:VA, :])
                rbps = ps_a.tile([D, 2 * QC], f32, tag="sa", name="rbps")
                for half in range(2):
                    hsl = slice(half * QC, (half + 1) * QC)
                    nc.tensor.matmul(
                        rbps[0:D, hsl], ones64[0:1, :],
                        rsh[0:1, hsl].bitcast(f32r),
                        start=True, stop=True,
                    )
                rn = p_norm.tile([D, 2 * QC], f32, tag="rn")
                nc.vector.reciprocal_approx_fast(out=rn[0:D, :], in_=rbps[0:D, :])
                nc.vector.tensor_mul(un_sb[j][0:D, cs], psu[0:D, 0:QC],
                                     rn[0:D, 0:QC])
                ut = p_norm.tile([D, QC], bf16, tag="ut")
                nc.vector.tensor_mul(ut[:], psu[0:D, QC:2 * QC],
                                     rn[0:D, QC:2 * QC])
                nc.gpsimd.dma_start(un_sb[j][D:PT, cs], ut[:])

            emit_norm(0, psus[0])
            if cq > 0:
                emit_outproj(cq - 1)    # fills the PE while norm j=1 runs
            emit_norm(1, psus[1])

        emit_outproj(NQC - 1)

    nc.compile()
    return nc


def _get_nc():
    if "nc" not in _CACHE:
        _CACHE["nc"] = _build_nc()
    return _CACHE["nc"]


def _make_in_maps(x, W_qkv, b_qkv, W_out, b_out):
    import ml_dtypes

    bf16 = ml_dtypes.bfloat16
    x = np.asarray(x, dtype=np.float32)
    W_qkv = np.asarray(W_qkv, dtype=np.float32)
    b_qkv = np.asarray(b_qkv, dtype=np.float32)
    W_out = np.asarray(W_out, dtype=np.float32)
    b_out = np.asarray(b_out, dtype=np.float32)

    i = np.arange(PT)[:, None]
    j = np.arange(PT)[None, :]
    trimask = (i <= j).astype(bf16)
    ident = np.eye(PT, dtype=bf16)
    ones64 = np.ones((1, D), dtype=np.float32)

    in_maps = []
    for c in range(N_CORES):
        b, g = divmod(c, 4)
        gs = slice(g * NC_, (g + 1) * NC_)
        # W_qkv columns for this head group: [C, 3*NC_]
        wqkv_c = np.concatenate(
            [W_qkv[:, gs], W_qkv[:, C:][:, gs], W_qkv[:, 2 * C:][:, gs]],
            axis=1)
        # repack as [128, KC * 3NC_] (k-chunk rows side by side)
        wq_r = np.ascontiguousarray(
            wqkv_c.reshape(KC, PT, 3 * NC_).transpose(1, 0, 2)
            .reshape(PT, KC * 3 * NC_)).astype(bf16)
        bq_r = np.ascontiguousarray(
            np.concatenate([b_qkv[gs], b_qkv[C:][gs], b_qkv[2 * C:][gs]])
            .reshape(MQKV, PT).T).astype(np.float32)
        # x^T repack: [C, T] -> [128, cq, k, 512]
        xT = x[b].T  # [C, T]
        xr = np.ascontiguousarray(
            xT.reshape(KC, PT, NQC, QC).transpose(1, 2, 0, 3)
            .reshape(PT, NQC * KC * QC)).astype(bf16)
        # W_out rows for this head group: [NC_, C] -> [128, 2*C]
        wo_r = np.ascontiguousarray(
            W_out[gs, :].reshape(2, PT, C).transpose(1, 0, 2)
            .reshape(PT, 2 * C)).astype(bf16)
        bo_r = np.ascontiguousarray(
            (b_out if g == 0 else np.zeros_like(b_out))
            .reshape(C // PT, PT).T).astype(np.float32)
        in_maps.append({
            "xr": xr,
            "wq": wq_r,
            "bq": bq_r,
            "wo": wo_r,
            "bo": bo_r,
            "trimask": trimask,
            "ident": ident,
            "ones64": ones64,
        })
    return in_maps


def _assemble(results):
    out = np.empty((B, T, C), dtype=np.float32)
    for b in range(B):
        acc = results[4 * b]["outT"].copy()
        for g in range(1, 4):
            acc += results[4 * b + g]["outT"]
        out[b] = acc.T
    return out


def kernel(x, W_qkv, b_qkv, W_out, b_out):
    from concourse import bass_utils
    nc = _get_nc()
    in_maps = _make_in_maps(x, W_qkv, b_qkv, W_out, b_out)
    res = bass_utils.run_bass_kernel_spmd(nc, in_maps, core_ids=list(range(N_CORES)))
    return _assemble(res.results)


# revision 7
# speedup vs baseline: 1.5543x; 1.0107x over previous
"""Causal self-attention (B=2, T=2048, C=1024, H=16, D=64) on 8 trn2 NeuronCores.

Sharding: core c = (batch b = c//4) x (head-group g = c%4; heads 4g..4g+3).
Tensor-parallel on heads for qkv_proj (column split) / out_proj (row split),
data-parallel on batch. Each core computes a full [C, T] partial of the
output projection for its batch; the host sums the 4 head-group partials
per batch and transposes back to [T, C].

v3: bf16 datapath (fp32 PSUM accumulate) with fused phase pipeline: the
QKV projection for chunk cq+1 is emitted between attention(cq) and its
normalize-finish, so the PE never idles across chunk boundaries and HAM
stays un-throttled. Normalize is split: start = l-row copy on the ACT
engine + partition shift on the gpsimd DMA queue (both off the PE/DVE
critical path), finish = rank-1 broadcast matmul (PE) +
reciprocal_approx_fast + scale (DVE). Out-proj is deferred one chunk and
stores bf16 partials (summed in fp32 on the host).
"""

import sys

if "/opt/trn_rl_repo" not in sys.path:
    sys.path.insert(0, "/opt/trn_rl_repo")

import numpy as np

B, T, C = 2, 2048, 1024
H, D = 16, 64
HPC = 4            # heads per core
NC_ = HPC * D      # 256 qkv columns per core per projection
N_CORES = 8
PT = 128           # partition tile
TT = T // PT       # 16 t tiles
QC = 512           # tq chunk (moving free dim)
NQC = T // QC      # 4 tq chunks
KC = C // PT       # 8 contraction chunks for qkv proj
MQKV = 3 * NC_ // PT  # 6 output row chunks of qkv proj
VA = D + 1         # v_aug cols per head

_CACHE = {}


def _build_nc():
    import concourse.bacc as bacc
    import concourse.mybir as mybir
    import concourse.tile as tile
    from contextlib import ExitStack

    f32 = mybir.dt.float32
    f32r = mybir.dt.float32r
    bf16 = mybir.dt.bfloat16
    Act = mybir.ActivationFunctionType

    nc = bacc.Bacc("TRN2", target_bir_lowering=False, debug=False,
                   num_devices=N_CORES)

    # Host-repacked inputs (see _make_in_maps):
    #   xr:  [128, NQC*KC*512] bf16 — x^T chunked [p, cq, k, 512]
    #   wq:  [128, KC*768]     bf16 — W_qkv chunk rows side by side
    #   wo:  [128, 2*1024]     bf16 — W_out chunk rows side by side
    xr_d = nc.dram_tensor("xr", [PT, NQC * KC * QC], bf16, kind="ExternalInput").ap()
    wq_d = nc.dram_tensor("wq", [PT, KC * 3 * NC_], bf16, kind="ExternalInput").ap()
    wo_d = nc.dram_tensor("wo", [PT, 2 * C], bf16, kind="ExternalInput").ap()
    bq_d = nc.dram_tensor("bq", [PT, MQKV], f32, kind="ExternalInput").ap()
    bo_d = nc.dram_tensor("bo", [PT, C // PT], f32, kind="ExternalInput").ap()
    trimask_d = nc.dram_tensor("trimask", [PT, PT], bf16, kind="ExternalInput").ap()
    ident_d = nc.dram_tensor("ident", [PT, PT], bf16, kind="ExternalInput").ap()
    ones64_d = nc.dram_tensor("ones64", [1, D], f32, kind="ExternalInput").ap()
    outT_d = nc.dram_tensor("outT", [C, T], bf16, kind="ExternalOutput").ap()

    with tile.TileContext(nc) as tc, ExitStack() as ctx:
        p_big = ctx.enter_context(tc.tile_pool(name="big", bufs=1))
        p_qk = ctx.enter_context(tc.tile_pool(name="qk", bufs=4))
        p_vt = ctx.enter_context(tc.tile_pool(name="vt", bufs=4))
        p_va = ctx.enter_context(tc.tile_pool(name="va", bufs=TT))
        p_small = ctx.enter_context(tc.tile_pool(name="small", bufs=1))
        p_pt = ctx.enter_context(tc.tile_pool(name="ptile", bufs=3))
        p_norm = ctx.enter_context(tc.tile_pool(name="norm", bufs=2))
        p_out = ctx.enter_context(tc.tile_pool(name="outs", bufs=2))
        ps_a = ctx.enter_context(tc.tile_pool(name="psa", bufs=2, space="PSUM"))
        ps_u = ctx.enter_context(tc.tile_pool(name="psu", bufs=2, space="PSUM"))

        # ---- loads (few, large DMAs; issue order = need order) ----------
        # x^T: SBUF layout [p, k, t] (k-major); DRAM layout [p, cq, k, 512]
        xt_sb = p_big.tile([PT, KC * T], bf16, tag="xt")
        xt3 = xt_sb.rearrange("p (k t) -> p k t", t=T)
        xr3 = xr_d.rearrange("p (ck q) -> p ck q", q=QC)
        nc.sync.dma_start(xt3[:, :, 0:QC], xr3[:, 0:KC, :])
        wq_sb = p_big.tile([PT, KC * 3 * NC_], bf16, tag="wq")
        nc.sync.dma_start(wq_sb[:, :KC * 3 * NC_ // 2], wq_d[:, :KC * 3 * NC_ // 2])
        nc.sync.dma_start(wq_sb[:, KC * 3 * NC_ // 2:], wq_d[:, KC * 3 * NC_ // 2:])
        bq_sb = p_small.tile([PT, MQKV], f32, tag="bq")
        nc.sync.dma_start(bq_sb[:], bq_d[:])
        for cq in range(1, NQC):
            nc.sync.dma_start(
                xt3[:, :, cq * QC:(cq + 1) * QC],
                xr3[:, cq * KC:(cq + 1) * KC, :],
            )

        ident = p_small.tile([PT, PT], bf16, tag="ident")
        nc.scalar.dma_start(ident[:], ident_d[:])
        trimask = p_small.tile([PT, PT], bf16, tag="trimask")
        nc.scalar.dma_start(trimask[:], trimask_d[:])
        ones64 = p_small.tile([1, D], f32r, tag="ones64")
        nc.scalar.dma_start(ones64[:], ones64_d[:].bitcast(f32r))
        wo_sb = p_big.tile([PT, 2 * C], bf16, tag="wo")
        nc.scalar.dma_start(wo_sb[:], wo_d[:])
        bo_sb = p_small.tile([PT, C // PT], f32, tag="bo")
        nc.scalar.dma_start(bo_sb[:], bo_d[:])

        # ---- persistent SBUF state --------------------------------------
        qk_sb = [p_qk.tile([PT, T], bf16, tag="qk", name=f"qk{j}") for j in range(4)]
        va_sb = [p_va.tile([PT, HPC * VA], bf16, tag="va", name=f"va{t}")
                 for t in range(TT)]
        for t in range(TT):
            nc.gpsimd.memset(
                va_sb[t].rearrange("p (h v) -> p h v", v=VA)[:, :, D:D + 1], 1.0)
        un_sb = [p_qk.tile([PT, T], bf16, tag="un", bufs=2, name=f"un{j}")
                 for j in range(2)]

        def alloc_ps(shape, name, dtype=f32):
            return ps_a.tile(shape, dtype, tag="sa", name=name)

        # ---- emitters ---------------------------------------------------
        def emit_qkv(cq):
            """QKV projection for tq chunk cq -> qk_sb columns, va tiles."""
            cs = slice(cq * QC, (cq + 1) * QC)
            for m in range(MQKV):
                ps = alloc_ps([PT, QC], "ps")
                for k in range(KC):
                    nc.tensor.matmul(
                        ps[:],
                        wq_sb[:, k * 3 * NC_ + m * PT:k * 3 * NC_ + (m + 1) * PT],
                        xt3[:, k, cs],
                        start=(k == 0), stop=(k == KC - 1),
                    )
                if m < 4:
                    nc.vector.tensor_scalar_add(qk_sb[m][:, cs], ps[:],
                                                bq_sb[:, m:m + 1])
                else:
                    h0 = 2 * (m - 4)
                    for q4 in range(4):
                        t = cq * 4 + q4
                        vtp = p_vt.tile([PT, PT], bf16, tag="vt")
                        nc.vector.tensor_scalar_add(
                            vtp[:], ps[:, q4 * PT:(q4 + 1) * PT],
                            bq_sb[:, m:m + 1])
                        pst = alloc_ps([PT, PT], "pst", bf16)
                        nc.tensor.transpose(pst[:], vtp[:], ident[:])
                        nc.vector.tensor_copy(
                            va_sb[t].rearrange("p (h v) -> p h v", v=VA)
                            [:, h0:h0 + 2, 0:D],
                            pst.rearrange("p (h v) -> p h v", v=D)[:, 0:2, :])

        def emit_attn(cq, j):
            """S^T = K Q^T, exp, mask, U^T accumulation for head pair j."""
            h0, h1 = 2 * j, 2 * j + 1
            nts = 4 * cq + 4
            psu = ps_u.tile([PT, 2 * QC], f32, tag="acc", name="psu")
            stage = []

            def drain():
                t, psS = stage.pop(0)
                p = t - 4 * cq
                s = max(p, 0) * PT
                pt = p_pt.tile([PT, 2 * QC], bf16, tag="pt")
                pt3 = pt.rearrange("p (h w) -> p h w", h=2)
                psS3 = psS.rearrange("p (h w) -> p h w", h=2)
                nc.scalar.activation(pt3[:, :, s:QC], psS3[:, :, s:QC],
                                     Act.Exp, scale=0.125)
                if p >= 0:
                    nc.vector.tensor_mul(
                        pt[:, s:s + PT], pt[:, s:s + PT], trimask[:])
                    nc.vector.tensor_mul(
                        pt[:, QC + s:QC + s + PT],
                        pt[:, QC + s:QC + s + PT], trimask[:])
                nc.tensor.matmul(
                    psu[0:VA, s:QC],
                    va_sb[t][:, h0 * VA:(h0 + 1) * VA], pt[:, s:QC],
                    start=(t == 0), stop=(t == nts - 1),
                )
                nc.tensor.matmul(
                    psu[0:VA, QC + s:2 * QC],
                    va_sb[t][:, h1 * VA:(h1 + 1) * VA],
                    pt[:, QC + s:2 * QC],
                    start=(t == 0), stop=(t == nts - 1),
                )

            for t in range(nts):
                p = t - 4 * cq      # >= 0 on diagonal-crossing tiles
                s = max(p, 0) * PT  # skip fully-masked leading columns
                psS = alloc_ps([PT, 2 * QC], "psS")
                tsl = slice(t * PT, (t + 1) * PT)
                qsl = slice(cq * QC + s, (cq + 1) * QC)
                nc.tensor.matmul(
                    psS[:, s:QC],
                    qk_sb[2 + j][0:D, tsl], qk_sb[j][0:D, qsl],
                    start=True, stop=True, tile_position=(0, 0),
                )
                nc.tensor.matmul(
                    psS[:, QC + s:2 * QC],
                    qk_sb[2 + j][D:PT, tsl], qk_sb[j][D:PT, qsl],
                    start=True, stop=True, tile_position=(D, 0),
                )
                stage.append((t, psS))
                if len(stage) > 1:
                    drain()
            drain()
            return psu

        def emit_norm_start(psu):
            """l-row to SBUF (ACT engine) and shift to partition 0 (gpsimd)."""
            rr = p_norm.tile([VA, 2 * QC], f32, tag="rr")
            nc.scalar.activation(rr[D:VA, :], psu[D:VA, :], Act.Copy)
            rsh = p_norm.tile([1, 2 * QC], f32, tag="rsh")
            nc.gpsimd.dma_start(rsh[0:1, :], rr[D:VA, :])
            return rsh

        def emit_norm_finish(cq, j, psu, rsh):
            """Broadcast l (PE), reciprocal + scale (DVE), h1 shift (gpsimd)."""
            cs = slice(cq * QC, (cq + 1) * QC)
            rbps = alloc_ps([D, 2 * QC], "rbps")
            for half in range(2):
                hsl = slice(half * QC, (half + 1) * QC)
                nc.tensor.matmul(
                    rbps[0:D, hsl], ones64[0:1, :],
                    rsh[0:1, hsl].bitcast(f32r),
                    start=True, stop=True,
                )
            rn = p_norm.tile([D, 2 * QC], f32, tag="rn")
            nc.vector.reciprocal_approx_fast(out=rn[0:D, :], in_=rbps[0:D, :])
            nc.vector.tensor_mul(un_sb[j][0:D, cs], psu[0:D, 0:QC],
                                 rn[0:D, 0:QC])
            ut = p_norm.tile([D, QC], bf16, tag="ut")
            nc.vector.tensor_mul(ut[:], psu[0:D, QC:2 * QC],
                                 rn[0:D, QC:2 * QC])
            nc.gpsimd.dma_start(un_sb[j][D:PT, cs], ut[:])

        def emit_outproj(cq):
            cs = slice(cq * QC, (cq + 1) * QC)
            for ep in range(4):
                pp2 = alloc_ps([PT, 2 * QC], "pp2")
                for half in range(2):
                    e = 2 * ep + half
                    for k in range(2):
                        nc.tensor.matmul(
                            pp2[:, half * QC:(half + 1) * QC],
                            wo_sb[:, k * C + e * PT:k * C + (e + 1) * PT],
                            un_sb[k][:, cs],
                            start=(k == 0), stop=(k == 1),
                        )
                ot = p_out.tile([PT, 2 * QC], bf16, tag="ot")
                for half in range(2):
                    e = 2 * ep + half
                    nc.vector.tensor_scalar_add(
                        ot[:, half * QC:(half + 1) * QC],
                        pp2[:, half * QC:(half + 1) * QC], bo_sb[:, e:e + 1])
                    nc.sync.dma_start(
                        outT_d[e * PT:(e + 1) * PT, cs],
                        ot[:, half * QC:(half + 1) * QC])

        # ---- fused pipeline ---------------------------------------------
        emit_qkv(0)
        for cq in range(NQC):
            psu0 = emit_attn(cq, 0)
            rsh0 = emit_norm_start(psu0)
            psu1 = emit_attn(cq, 1)
            rsh1 = emit_norm_start(psu1)
            if cq > 0:
                emit_outproj(cq - 1)
            if cq + 1 < NQC:
                emit_qkv(cq + 1)
            emit_norm_finish(cq, 0, psu0, rsh0)
            emit_norm_finish(cq, 1, psu1, rsh1)
        emit_outproj(NQC - 1)

    nc.compile()
    return nc


def _get_nc():
    if "nc" not in _CACHE:
        _CACHE["nc"] = _build_nc()
    return _CACHE["nc"]


def _make_in_maps(x, W_qkv, b_qkv, W_out, b_out):
    import ml_dtypes

    bf16 = ml_dtypes.bfloat16
    x = np.asarray(x, dtype=np.float32)
    W_qkv = np.asarray(W_qkv, dtype=np.float32)
    b_qkv = np.asarray(b_qkv, dtype=np.float32)
    W_out = np.asarray(W_out, dtype=np.float32)
    b_out = np.asarray(b_out, dtype=np.float32)

    i = np.arange(PT)[:, None]
    j = np.arange(PT)[None, :]
    trimask = (i <= j).astype(bf16)
    ident = np.eye(PT, dtype=bf16)
    ones64 = np.ones((1, D), dtype=np.float32)

    in_maps = []
    for c in range(N_CORES):
        b, g = divmod(c, 4)
        gs = slice(g * NC_, (g + 1) * NC_)
        # W_qkv columns for this head group: [C, 3*NC_]
        wqkv_c = np.concatenate(
            [W_qkv[:, gs], W_qkv[:, C:][:, gs], W_qkv[:, 2 * C:][:, gs]],
            axis=1)
        # repack as [128, KC * 3NC_] (k-chunk rows side by side)
        wq_r = np.ascontiguousarray(
            wqkv_c.reshape(KC, PT, 3 * NC_).transpose(1, 0, 2)
            .reshape(PT, KC * 3 * NC_)).astype(bf16)
        bq_r = np.ascontiguousarray(
            np.concatenate([b_qkv[gs], b_qkv[C:][gs], b_qkv[2 * C:][gs]])
            .reshape(MQKV, PT).T).astype(np.float32)
        # x^T repack: [C, T] -> [128, cq, k, 512]
        xT = x[b].T  # [C, T]
        xr = np.ascontiguousarray(
            xT.reshape(KC, PT, NQC, QC).transpose(1, 2, 0, 3)
            .reshape(PT, NQC * KC * QC)).astype(bf16)
        # W_out rows for this head group: [NC_, C] -> [128, 2*C]
        wo_r = np.ascontiguousarray(
            W_out[gs, :].reshape(2, PT, C).transpose(1, 0, 2)
            .reshape(PT, 2 * C)).astype(bf16)
        bo_r = np.ascontiguousarray(
            (b_out if g == 0 else np.zeros_like(b_out))
            .reshape(C // PT, PT).T).astype(np.float32)
        in_maps.append({
            "xr": xr,
            "wq": wq_r,
            "bq": bq_r,
            "wo": wo_r,
            "bo": bo_r,
            "trimask": trimask,
            "ident": ident,
            "ones64": ones64,
        })
    return in_maps


def _assemble(results):
    out = np.empty((B, T, C), dtype=np.float32)
    for b in range(B):
        acc = results[4 * b]["outT"].astype(np.float32)
        for g in range(1, 4):
            acc += results[4 * b + g]["outT"].astype(np.float32)
        out[b] = acc.T
    return out


def kernel(x, W_qkv, b_qkv, W_out, b_out):
    from concourse import bass_utils
    nc = _get_nc()
    in_maps = _make_in_maps(x, W_qkv, b_qkv, W_out, b_out)
    res = bass_utils.run_bass_kernel_spmd(nc, in_maps, core_ids=list(range(N_CORES)))
    return _assemble(res.results)
